# revision 17
# baseline (speedup 1.0000x reference)
"""Fused multi-head attention + residual + LayerNorm for TRN2, 8 NeuronCores.

Problem (B=2, S=2048, D=1024, H=16, HD=64):
  qh = (q @ Wq.T + bq)  (same k, v)
  scores = qh kh^T / sqrt(HD), masked, softmax -> attention (returned!)
  context = attention @ vh ; out = context @ Wo.T + bo
  output = LayerNorm(out + q residual) * gamma + beta
  returns (output, attention)

Sharding: data-parallel over (batch, query-row-block): core c handles batch
c//4, query rows (c%4)*512 ... +512, ALL heads.  K/V projections for the
core's batch are computed redundantly on the 4 cores sharing that batch --
this removes all cross-core communication (no collectives at all).

Device notes:
 - All matmuls in float32r (fp32, 11-bit mantissa, full PE rate). Matmul
   operands must be produced by compute ops with f32r output dtype.
 - Scores are computed TRANSPOSED: st[j, i], so softmax's sum over j is done
   by the PE via a ones-row appended to V (Vp[:, h, 64] == 1).  exp() runs on
   ACT straight out of PSUM with the 1/8 scale folded in.
 - Back-to-back matmuls into the SAME PSUM bank run ~2x slow, so all
   accumulation chains are emitted as interleaved bank pairs, and the PV
   chain of head i-1 is interleaved with the scores stream of head i.
 - Projected K is spilled to HBM and streamed back per head-pair (SBUF is too
   small to hold K^T, V and all working tiles at once).
 - Attention probs are written transposed (attnT[h, j, i]); the host
   transposes while unsharding.
"""
import numpy as np

import concourse.bacc as bacc
import concourse.tile as tile
from concourse import mybir
from concourse.bass_utils import run_bass_kernel_spmd

B, S, D, H = 2, 2048, 1024, 16
HD = D // H  # 64
NCORES = 8
RB = 512  # query rows per core
EPS = 1e-5
NKC = D // 128  # 8 contraction chunks
NHP = H // 2  # 8 head pairs
NJC = S // 128  # 16 key chunks of 128
NIC = RB // 128  # 4 row chunks

F32 = mybir.dt.float32
F32R = mybir.dt.float32r
BF16 = mybir.dt.bfloat16
I32 = mybir.dt.int32
AF = mybir.ActivationFunctionType
ALU = mybir.AluOpType

_nc_cache = {}


def _build(masked: bool):
    nc = bacc.Bacc("TRN2", target_bir_lowering=False, debug=False, num_devices=NCORES)

    xqT = nc.declare_dram_parameter("xqT", [D, RB], F32, isOutput=False)
    res = nc.declare_dram_parameter("res", [RB, D], F32, isOutput=False)
    xkT = nc.declare_dram_parameter("xkT", [D, S], F32, isOutput=False)
    xvT = nc.declare_dram_parameter("xvT", [D, S], F32, isOutput=False)
    wqT = nc.declare_dram_parameter("wqT", [D, D], F32, isOutput=False)
    wkT = nc.declare_dram_parameter("wkT", [D, D], F32, isOutput=False)
    wvT = nc.declare_dram_parameter("wvT", [D, D], F32, isOutput=False)
    woT = nc.declare_dram_parameter("woT", [D, D], F32, isOutput=False)
    bq2 = nc.declare_dram_parameter("bq2", [128, NKC], F32, isOutput=False)
    bk2 = nc.declare_dram_parameter("bk2", [128, NKC], F32, isOutput=False)
    bvr = nc.declare_dram_parameter("bvr", [1, D], F32, isOutput=False)
    bor = nc.declare_dram_parameter("bor", [1, D], F32, isOutput=False)
    gmr = nc.declare_dram_parameter("gmr", [1, D], F32, isOutput=False)
    btr = nc.declare_dram_parameter("btr", [1, D], F32, isOutput=False)
    if masked:
        mskT = nc.declare_dram_parameter("mskT", [S, RB], I32, isOutput=False)

    attnT = nc.declare_dram_parameter("attnT", [H, S, RB], F32, isOutput=True)
    outp = nc.declare_dram_parameter("outp", [RB, D], F32, isOutput=True)

    with tile.TileContext(nc) as tc:
        with (
            tc.tile_pool(name="consts", bufs=1) as consts,
            tc.tile_pool(name="stage", bufs=4) as stage,  # [128,512] f32 staging
            tc.tile_pool(name="rowp", bufs=1) as rowp,  # [1,D] rows
            tc.tile_pool(name="qt", bufs=1) as qtp,  # 8 tags [128,512] f32r
            tc.tile_pool(name="vp", bufs=1) as vpp,  # 16 tags [128,16,65] f32r
            tc.tile_pool(name="cx", bufs=1) as cxp,  # 8 tags [128,512] f32r
            tc.tile_pool(name="pn", bufs=2) as pnp,
            tc.tile_pool(name="small", bufs=2) as small,
            tc.tile_pool(name="dram", bufs=1, space="DRAM") as dramp,
            tc.tile_pool(name="pp", bufs=2, space="PSUM") as pp,
            tc.tile_pool(name="stp", bufs=2, space="PSUM") as stpp,
            tc.tile_pool(name="cxps", bufs=2, space="PSUM") as cxps,
        ):
            # ---- constants ----
            bq_t = consts.tile([128, NKC], F32, tag="bq")
            bk_t = consts.tile([128, NKC], F32, tag="bk")
            nc.sync.dma_start(out=bq_t, in_=bq2[:, :])
            nc.sync.dma_start(out=bk_t, in_=bk2[:, :])
            eps_t = consts.tile([128, 1], F32, tag="eps")
            nc.vector.memset(eps_t, EPS)
            ones16f = consts.tile([128, 16], F32, tag="o16f")
            nc.vector.memset(ones16f, 1.0)
            ones16r = consts.tile([128, 16], F32R, tag="o16r")
            nc.vector.tensor_copy(ones16r[:, :], ones16f[:, :])

            ktdram = dramp.tile([NHP, 128, S], F32R, tag="ktd")

            def stage_row_bcast(param, pool, tag):
                t = pool.tile([128, D], F32, tag=tag)
                rw = rowp.tile([1, D], F32, tag="row")
                nc.sync.dma_start(out=rw, in_=param[:, :])
                nc.gpsimd.partition_broadcast(t[:, :], rw[:, :])
                return t

            def load_wslice(wparam, kc, hp, pool, round_eng):
                st = stage.tile([128, 128], F32, tag="wst")
                nc.sync.dma_start(
                    out=st,
                    in_=wparam[kc * 128 : (kc + 1) * 128, hp * 128 : (hp + 1) * 128],
                )
                wr = pool.tile([128, 128], F32R, tag="wsl")
                if round_eng == "v":
                    nc.vector.tensor_copy(wr[:, :], st[:, :])
                else:
                    nc.gpsimd.tensor_copy(out=wr[:, :], in_=st[:, :])
                return wr

            # ================= Q projection =================
            # Qt[hp] = [128(d of 2 heads), 512(i)] = Wq[dslice,:] @ q_rows^T
            # head-pair groups interleaved in bank pairs
            Qt = [None] * NHP
            with (
                tc.tile_pool(name="xq", bufs=1) as xqpool,
                tc.tile_pool(name="wsl", bufs=32) as wslpool,
            ):
                xq_r = []
                for kc in range(NKC):
                    st = stage.tile([128, RB], F32, tag="stg")
                    nc.sync.dma_start(out=st, in_=xqT[kc * 128 : (kc + 1) * 128, :])
                    xr = xqpool.tile([128, RB], F32R, tag=f"xq{kc}")
                    nc.vector.tensor_copy(xr[:, :], st[:, :])
                    xq_r.append(xr)
                wslq = {
                    hp: [load_wslice(wqT, kc, hp, wslpool, "v") for kc in range(NKC)]
                    for hp in (0, 1)
                }
                for hpb in range(NHP // 2):
                    hps = (2 * hpb, 2 * hpb + 1)
                    if hpb + 1 < NHP // 2:
                        for hp in (2 * hpb + 2, 2 * hpb + 3):
                            wslq[hp] = [
                                load_wslice(wqT, kc, hp, wslpool, "v")
                                for kc in range(NKC)
                            ]
                    wsl = wslq
                    pss = {hp: pp.tile([128, RB], F32, tag="pp", name=f"psq{hp}") for hp in hps}
                    for kc in range(NKC):
                        for hp in hps:
                            nc.tensor.matmul(
                                pss[hp][:, :],
                                wsl[hp][kc][:, :],
                                xq_r[kc][:, :],
                                start=(kc == 0),
                                stop=(kc == NKC - 1),
                            )
                    for hp in hps:
                        qt = qtp.tile([128, RB], F32R, tag=f"qt{hp}")
                        nc.vector.tensor_scalar(
                            qt[:, :], pss[hp][:, :], bq_t[:, hp : hp + 1], None, ALU.add
                        )
                        Qt[hp] = qt

            # ================= V projection =================
            # V in natural [j, d] layout + ones column: vp[jc]=[128(j),16(h),65]
            # per jc: the two dc-halves form an interleaved bank pair
            Vp = []
            for jc in range(NJC):
                vp = vpp.tile([128, H, HD + 1], F32R, tag=f"vp{jc}")
                nc.vector.tensor_copy(vp[:, :, HD], ones16r[:, :])
                Vp.append(vp)
            with (
                tc.tile_pool(name="wv", bufs=1) as wvpool,
                tc.tile_pool(name="xvsl", bufs=16) as xvslpool,
                tc.tile_pool(name="bvp", bufs=1) as bvpool,
            ):
                bv_bc = stage_row_bcast(bvr, bvpool, "bvbc")
                wv_r = []
                for kc in range(NKC):
                    wr = wvpool.tile([128, D], F32R, tag=f"wv{kc}")
                    for piece in range(2):
                        st = stage.tile([128, 512], F32, tag="stg")
                        nc.sync.dma_start(
                            out=st,
                            in_=wvT[
                                kc * 128 : (kc + 1) * 128,
                                piece * 512 : (piece + 1) * 512,
                            ],
                        )
                        nc.scalar.activation(
                            wr[:, piece * 512 : (piece + 1) * 512], st[:, :], AF.Copy
                        )
                    wv_r.append(wr)
                def load_xvsl(jc):
                    out = []
                    for kc in range(NKC):
                        st = stage.tile([128, 128], F32, tag="wst")
                        nc.sync.dma_start(
                            out=st,
                            in_=xvT[
                                kc * 128 : (kc + 1) * 128, jc * 128 : (jc + 1) * 128
                            ],
                        )
                        xr = xvslpool.tile([128, 128], F32R, tag="xvsl")
                        nc.vector.tensor_copy(xr[:, :], st[:, :])
                        out.append(xr)
                    return out

                xvnext = load_xvsl(0)
                for jc in range(NJC):
                    xvsl = xvnext
                    if jc + 1 < NJC:
                        xvnext = load_xvsl(jc + 1)
                    pss = [pp.tile([128, 512], F32, tag="pp", name=f"psv{i}") for i in range(2)]
                    for kc in range(NKC):
                        for dc in range(2):
                            nc.tensor.matmul(
                                pss[dc][:, :],
                                xvsl[kc][:, :],
                                wv_r[kc][:, dc * 512 : (dc + 1) * 512],
                                start=(kc == 0),
                                stop=(kc == NKC - 1),
                            )
                    for dc in range(2):
                        nc.vector.tensor_tensor(
                            Vp[jc][:, dc * 8 : (dc + 1) * 8, 0:HD],
                            pss[dc][:, :].rearrange("p (h d) -> p h d", h=8),
                            bv_bc[:, dc * 512 : (dc + 1) * 512].rearrange(
                                "p (h d) -> p h d", h=8
                            ),
                            ALU.add,
                        )

            # ================= K projection (spilled to HBM) =================
            with (
                tc.tile_pool(name="xk", bufs=1) as xkpool,
                tc.tile_pool(name="wslk", bufs=16) as wslkpool,
                tc.tile_pool(name="kev", bufs=2) as kevpool,
            ):
                xk_r = []
                for kc in range(NKC):
                    xr = xkpool.tile([128, S], F32R, tag=f"xk{kc}")
                    for piece in range(4):
                        st = stage.tile([128, 512], F32, tag="stg")
                        nc.sync.dma_start(
                            out=st,
                            in_=xkT[
                                kc * 128 : (kc + 1) * 128,
                                piece * 512 : (piece + 1) * 512,
                            ],
                        )
                        if piece % 2 == 0:
                            nc.vector.tensor_copy(
                                xr[:, piece * 512 : (piece + 1) * 512], st[:, :]
                            )
                        else:
                            nc.scalar.activation(
                                xr[:, piece * 512 : (piece + 1) * 512],
                                st[:, :],
                                AF.Copy,
                            )
                    xk_r.append(xr)
                wslk = {
                    0: [load_wslice(wkT, kc, 0, wslkpool, "v") for kc in range(NKC)]
                }
                for hp in range(NHP):
                    if hp + 1 < NHP:
                        wslk[hp + 1] = [
                            load_wslice(wkT, kc, hp + 1, wslkpool, "v")
                            for kc in range(NKC)
                        ]
                    wsl = wslk.pop(hp)
                    # interleave the 4 j4-groups as 2 bank pairs
                    for j4p in range(2):
                        j4s = (2 * j4p, 2 * j4p + 1)
                        pss = {j4: pp.tile([128, 512], F32, tag="pp", name=f"psk{j4}") for j4 in j4s}
                        for kc in range(NKC):
                            for j4 in j4s:
                                nc.tensor.matmul(
                                    pss[j4][:, :],
                                    wsl[kc][:, :],
                                    xk_r[kc][:, j4 * 512 : (j4 + 1) * 512],
                                    start=(kc == 0),
                                    stop=(kc == NKC - 1),
                                )
                        for j4 in j4s:
                            kev = kevpool.tile([128, 512], F32R, tag="kev")
                            nc.vector.tensor_scalar(
                                kev[:, :],
                                pss[j4][:, :],
                                bk_t[:, hp : hp + 1],
                                None,
                                ALU.add,
                            )
                            nc.sync.dma_start(
                                out=ktdram[hp, :, j4 * 512 : (j4 + 1) * 512],
                                in_=kev[:, :],
                            )

            # ================= attention =================
            # software pipeline over heads: PV chain of head i-1 is interleaved
            # with the scores/exp stream of head i (alternating PSUM banks).
            CtxP = [None] * NHP
            with (
                tc.tile_pool(name="ktr", bufs=2) as ktrpool,
                tc.tile_pool(name="pt", bufs=(5 if masked else 6)) as ptp,
                tc.tile_pool(name="lin", bufs=2) as linp,
                tc.tile_pool(name="mbp", bufs=1) as mbp,
            ):
                mb = None
                if masked:
                    mb = []
                    for jc in range(NJC):
                        mi = stage.tile([128, RB], I32, tag="stg")
                        nc.sync.dma_start(
                            out=mi, in_=mskT[jc * 128 : (jc + 1) * 128, :]
                        )
                        mt = mbp.tile([128, RB], BF16, tag=f"mb{jc}")
                        nc.vector.tensor_scalar(
                            mt[:, :], mi[:, :], 1e9, 1e9, ALU.mult, ALU.subtract
                        )
                        mb.append(mt)

                def load_ktr(hp):
                    ktr = ktrpool.tile([128, S], F32R, tag="ktr")
                    for piece in range(4):
                        st = stage.tile([128, 512], F32, tag="stg")
                        nc.scalar.dma_start(
                            out=st,
                            in_=ktdram[hp, :, piece * 512 : (piece + 1) * 512].bitcast(
                                F32
                            ),
                        )
                        nc.gpsimd.tensor_copy(
                            out=ktr[:, piece * 512 : (piece + 1) * 512],
                            in_=st[:, :],
                        )
                    return ktr

                def emit_tail(state):
                    """l -> 1/l -> broadcast; normalize ctx + probs; DMA out."""
                    h, po, cps, pts, ctx_packed = state
                    linv = small.tile([1, RB], F32, tag="linv")
                    nc.vector.reciprocal(linv[:, :], cps[HD : HD + 1, :])
                    linb = linp.tile([128, RB], F32, tag="linb")
                    nc.gpsimd.partition_broadcast(linb[:, :], linv[:, :])
                    nc.vector.tensor_mul(
                        ctx_packed[po : po + 64, :], cps[0:HD, :], linb[0:HD, :]
                    )
                    for jq in range(NJC // 4):
                        pn = pnp.tile([128, 4, RB], F32, tag="pn")
                        eng = nc.gpsimd if (jq == 1) else nc.vector
                        eng.tensor_mul(
                            pn[:, :, :],
                            pts[jq][:, :, :].bitcast(F32),
                            linb[:, :].rearrange("p (u i) -> p u i", u=1).broadcast_to(
                                [128, 4, RB]
                            ),
                        )
                        nc.sync.dma_start(
                            out=attnT[h, jq * 512 : (jq + 1) * 512, :].rearrange(
                                "(u p) i -> p u i", u=4
                            ),
                            in_=pn[:, :, :],
                        )

                NP2 = NJC // 2  # 8 jc-pairs per head
                ktr_tiles = {}
                for idx in range(H):
                    hp, h2 = divmod(idx, 2)
                    if h2 == 0:
                        if hp == 0:
                            ktr_tiles[0] = load_ktr(0)
                        ctx_packed = cxp.tile([128, RB], F32R, tag=f"cxp{hp}")
                        CtxP[hp] = ctx_packed
                    ktr = ktr_tiles[hp]
                    po = h2 * 64
                    cps = cxps.tile([128, RB], F32, tag="cxps")
                    pts = []  # pt pair tiles

                    def pv(jp):
                        jq, half = divmod(jp, 2)
                        for sub in range(2):
                            jc = 2 * jp + sub
                            nc.tensor.matmul(
                                cps[0 : HD + 1, :],
                                Vp[jc][:, idx, :],
                                pts[jq][:, 2 * half + sub, :],
                                start=(jc == 0),
                                stop=(jc == NJC - 1),
                            )

                    for jp in range(NP2):
                        jq, half = divmod(jp, 2)
                        if half == 0:
                            ptq = ptp.tile([128, 4, RB], F32R, tag="pt", name="ptq")
                            pts.append(ptq)
                        st_ps = stpp.tile([128, 2, RB], F32, tag="stp")
                        for sub in range(2):
                            jc = 2 * jp + sub
                            nc.tensor.matmul(
                                st_ps[:, sub, :],
                                ktr[po : po + 64, jc * 128 : (jc + 1) * 128],
                                Qt[hp][po : po + 64, :],
                                start=True,
                                stop=True,
                            )
                        if masked:
                            for sub in range(2):
                                nc.vector.tensor_tensor(
                                    st_ps[:, sub, :],
                                    st_ps[:, sub, :],
                                    mb[2 * jp + sub][:, :],
                                    ALU.add,
                                )
                        nc.scalar.activation(
                            pts[jq][:, 2 * half : 2 * half + 2, :],
                            st_ps[:, :, :],
                            AF.Exp,
                            scale=0.125,
                        )
                        if jp >= 1:
                            pv(jp - 1)
                    pv(NP2 - 1)
                    emit_tail((idx, po, cps, pts, CtxP[hp]))
                    # prefetch next pair's K mid-pair
                    if h2 == 0 and hp + 1 < NHP:
                        ktr_tiles[hp + 1] = load_ktr(hp + 1)

            # ================= output proj + layernorm =================
            with (
                tc.tile_pool(name="wo", bufs=1) as wopool,
                tc.tile_pool(name="bcs", bufs=1) as bcpool,
                tc.tile_pool(name="xt", bufs=2) as xtpool,
            ):
                bo_bc = stage_row_bcast(bor, bcpool, "bobc")
                gm_bc = stage_row_bcast(gmr, bcpool, "gmbc")
                bt_bc = stage_row_bcast(btr, bcpool, "btbc")
                wo_r = []
                for kc in range(NKC):
                    wr = wopool.tile([128, D], F32R, tag=f"wo{kc}")
                    for piece in range(2):
                        st = stage.tile([128, 512], F32, tag="stg")
                        nc.scalar.dma_start(
                            out=st,
                            in_=woT[
                                kc * 128 : (kc + 1) * 128,
                                piece * 512 : (piece + 1) * 512,
                            ],
                        )
                        nc.scalar.activation(
                            wr[:, piece * 512 : (piece + 1) * 512], st[:, :], AF.Copy
                        )
                    wo_r.append(wr)

                for ic in range(NIC):
                    xt = xtpool.tile([128, D], F32, tag="xt")
                    pss = [pp.tile([128, 512], F32, tag="pp", name=f"psv{i}") for i in range(2)]
                    for hp in range(NHP):
                        for dc in range(2):
                            nc.tensor.matmul(
                                pss[dc][:, :],
                                CtxP[hp][:, ic * 128 : (ic + 1) * 128],
                                wo_r[hp][:, dc * 512 : (dc + 1) * 512],
                                start=(hp == 0),
                                stop=(hp == NHP - 1),
                            )
                    for dc in range(2):
                        rst = stage.tile([128, 512], F32, tag="stg")
                        nc.sync.dma_start(
                            out=rst,
                            in_=res[
                                ic * 128 : (ic + 1) * 128, dc * 512 : (dc + 1) * 512
                            ],
                        )
                        nc.vector.tensor_tensor(
                            xt[:, dc * 512 : (dc + 1) * 512],
                            pss[dc][:, :],
                            rst[:, :],
                            ALU.add,
                        )
                    nc.vector.tensor_tensor(xt[:, :], xt[:, :], bo_bc[:, :], ALU.add)
                    stats = small.tile([128, 2, 6], F32, tag="stats")
                    for sg in range(2):
                        nc.vector.bn_stats(
                            stats[:, sg, :], xt[:, sg * 512 : (sg + 1) * 512]
                        )
                    mv = small.tile([128, 2], F32, tag="mv")
                    nc.vector.bn_aggr(mv[:, :], stats[:, :, :])
                    sd = small.tile([128, 1], F32, tag="sd")
                    nc.scalar.activation(
                        sd[:, :], mv[:, 1:2], AF.Sqrt, bias=eps_t[:, 0:1]
                    )
                    rstd = small.tile([128, 1], F32, tag="rstd")
                    nc.vector.reciprocal(rstd[:, :], sd[:, :])
                    nc.vector.tensor_scalar(
                        xt[:, :],
                        xt[:, :],
                        mv[:, 0:1],
                        rstd[:, 0:1],
                        ALU.subtract,
                        ALU.mult,
                    )
                    nc.vector.tensor_tensor(xt[:, :], xt[:, :], gm_bc[:, :], ALU.mult)
                    nc.vector.tensor_tensor(xt[:, :], xt[:, :], bt_bc[:, :], ALU.add)
                    nc.sync.dma_start(
                        out=outp[ic * 128 : (ic + 1) * 128, :], in_=xt[:, :]
                    )

    nc.finalize()
    return nc


def get_nc(masked: bool):
    if masked not in _nc_cache:
        _nc_cache[masked] = _build(masked)
    return _nc_cache[masked]


def make_in_maps(q, k, v, mask, Wq, bq, Wk, bk, Wv, bv, Wo, bo, gamma, beta, masked):
    f32 = np.float32
    kT = [np.ascontiguousarray(k[b].T).astype(f32, copy=False) for b in range(B)]
    vT = [np.ascontiguousarray(v[b].T).astype(f32, copy=False) for b in range(B)]
    wqT = np.ascontiguousarray(np.asarray(Wq, f32).T)
    wkT = np.ascontiguousarray(np.asarray(Wk, f32).T)
    wvT = np.ascontiguousarray(np.asarray(Wv, f32).T)
    woT = np.ascontiguousarray(np.asarray(Wo, f32).T)
    bq2 = np.ascontiguousarray(np.asarray(bq, f32).reshape(NKC, 128).T)
    bk2 = np.ascontiguousarray(np.asarray(bk, f32).reshape(NKC, 128).T)
    bvr = np.ascontiguousarray(np.asarray(bv, f32).reshape(1, D))
    bor = np.ascontiguousarray(np.asarray(bo, f32).reshape(1, D))
    gmr = np.ascontiguousarray(np.asarray(gamma, f32).reshape(1, D))
    btr = np.ascontiguousarray(np.asarray(beta, f32).reshape(1, D))
    in_maps = []
    for c in range(NCORES):
        b, r = divmod(c, NCORES // B)
        rows = slice(r * RB, (r + 1) * RB)
        m = {
            "xqT": np.ascontiguousarray(q[b, rows, :].T),
            "res": np.ascontiguousarray(q[b, rows, :]),
            "xkT": kT[b],
            "xvT": vT[b],
            "wqT": wqT,
            "wkT": wkT,
            "wvT": wvT,
            "woT": woT,
            "bq2": bq2,
            "bk2": bk2,
            "bvr": bvr,
            "bor": bor,
            "gmr": gmr,
            "btr": btr,
        }
        if masked:
            m["mskT"] = np.ascontiguousarray(mask[b, rows, :].T).astype(
                np.int32, copy=False
            )
        in_maps.append(m)
    return in_maps


def assemble(results):
    output = np.empty((B, S, D), np.float32)
    attention = np.empty((B, H, S, S), np.float32)
    for c in range(NCORES):
        b, r = divmod(c, NCORES // B)
        rows = slice(r * RB, (r + 1) * RB)
        output[b, rows, :] = results[c]["outp"]
        attention[b, :, rows, :] = results[c]["attnT"].transpose(0, 2, 1)
    return output, attention


def kernel(q, k, v, mask, Wq, bq, Wk, bk, Wv, bv, Wo, bo, gamma, beta, **run_kwargs):
    q = np.ascontiguousarray(np.asarray(q), dtype=np.float32)
    k = np.ascontiguousarray(np.asarray(k), dtype=np.float32)
    v = np.ascontiguousarray(np.asarray(v), dtype=np.float32)
    mask = np.asarray(mask)
    masked = bool((np.asarray(mask) == 0).any())
    nc = get_nc(masked)
    in_maps = make_in_maps(
        q, k, v, mask, Wq, bq, Wk, bk, Wv, bv, Wo, bo, gamma, beta, masked
    )
    res = run_bass_kernel_spmd(nc, in_maps, list(range(NCORES)), **run_kwargs)
    output, attention = assemble(res.results)
    if run_kwargs:
        return (output, attention), res
    return output, attention


# revision 18
# speedup vs baseline: 1.0111x; 1.0111x over previous
"""Fused multi-head attention + residual + LayerNorm for TRN2, 8 NeuronCores.

Problem (B=2, S=2048, D=1024, H=16, HD=64):
  qh = (q @ Wq.T + bq)  (same k, v)
  scores = qh kh^T / sqrt(HD), masked, softmax -> attention (returned!)
  context = attention @ vh ; out = context @ Wo.T + bo
  output = LayerNorm(out + q residual) * gamma + beta
  returns (output, attention)

Sharding: data-parallel over (batch, query-row-block): core c handles batch
c//4, query rows (c%4)*512 ... +512, ALL heads.  K/V projections for the
core's batch are computed redundantly on the 4 cores sharing that batch --
this removes all cross-core communication (no collectives at all).

Device notes:
 - All matmuls in float32r (fp32, 11-bit mantissa, full PE rate). Matmul
   operands must be produced by compute ops with f32r output dtype.
 - Scores are computed TRANSPOSED: st[j, i], so softmax's sum over j is done
   by the PE via a ones-row appended to V (Vp[:, h, 64] == 1).  exp() runs on
   ACT straight out of PSUM with the 1/8 scale folded in.
 - Back-to-back matmuls into the SAME PSUM bank run ~2x slow, so all
   accumulation chains are emitted as interleaved bank pairs, and the PV
   chain of head i-1 is interleaved with the scores stream of head i.
 - Projected K is spilled to HBM and streamed back per head-pair (SBUF is too
   small to hold K^T, V and all working tiles at once).
 - Attention probs are written transposed (attnT[h, j, i]); the host
   transposes while unsharding.
"""
import numpy as np

import concourse.bacc as bacc
import concourse.tile as tile
from concourse import mybir
from concourse.bass_utils import run_bass_kernel_spmd

B, S, D, H = 2, 2048, 1024, 16
HD = D // H  # 64
NCORES = 8
RB = 512  # query rows per core
EPS = 1e-5
NKC = D // 128  # 8 contraction chunks
NHP = H // 2  # 8 head pairs
NJC = S // 128  # 16 key chunks of 128
NIC = RB // 128  # 4 row chunks

F32 = mybir.dt.float32
F32R = mybir.dt.float32r
BF16 = mybir.dt.bfloat16
I32 = mybir.dt.int32
AF = mybir.ActivationFunctionType
ALU = mybir.AluOpType

_nc_cache = {}


def _build(masked: bool):
    nc = bacc.Bacc("TRN2", target_bir_lowering=False, debug=False, num_devices=NCORES)

    xqT = nc.declare_dram_parameter("xqT", [D, RB], F32, isOutput=False)
    res = nc.declare_dram_parameter("res", [RB, D], F32, isOutput=False)
    xkT = nc.declare_dram_parameter("xkT", [D, S], F32, isOutput=False)
    xvT = nc.declare_dram_parameter("xvT", [D, S], F32, isOutput=False)
    wqT = nc.declare_dram_parameter("wqT", [D, D], F32, isOutput=False)
    wkT = nc.declare_dram_parameter("wkT", [D, D], F32, isOutput=False)
    wvT = nc.declare_dram_parameter("wvT", [D, D], F32, isOutput=False)
    woT = nc.declare_dram_parameter("woT", [D, D], F32, isOutput=False)
    bq2 = nc.declare_dram_parameter("bq2", [128, NKC], F32, isOutput=False)
    bk2 = nc.declare_dram_parameter("bk2", [128, NKC], F32, isOutput=False)
    bvr = nc.declare_dram_parameter("bvr", [1, D], F32, isOutput=False)
    bor = nc.declare_dram_parameter("bor", [1, D], F32, isOutput=False)
    gmr = nc.declare_dram_parameter("gmr", [1, D], F32, isOutput=False)
    btr = nc.declare_dram_parameter("btr", [1, D], F32, isOutput=False)
    if masked:
        mskT = nc.declare_dram_parameter("mskT", [S, RB], I32, isOutput=False)

    attnT = nc.declare_dram_parameter("attnT", [H, S, RB], F32, isOutput=True)
    outp = nc.declare_dram_parameter("outp", [RB, D], F32, isOutput=True)

    with tile.TileContext(nc) as tc:
        with (
            tc.tile_pool(name="consts", bufs=1) as consts,
            tc.tile_pool(name="stage", bufs=4) as stage,  # [128,512] f32 staging
            tc.tile_pool(name="rowp", bufs=1) as rowp,  # [1,D] rows
            tc.tile_pool(name="qt", bufs=1) as qtp,  # 8 tags [128,512] f32r
            tc.tile_pool(name="vp", bufs=1) as vpp,  # 16 tags [128,16,65] f32r
            tc.tile_pool(name="cx", bufs=1) as cxp,  # 8 tags [128,512] f32r
            tc.tile_pool(name="pn", bufs=2) as pnp,
            tc.tile_pool(name="small", bufs=2) as small,
            tc.tile_pool(name="dram", bufs=1, space="DRAM") as dramp,
            tc.tile_pool(name="pp", bufs=2, space="PSUM") as pp,
            tc.tile_pool(name="stp", bufs=2, space="PSUM") as stpp,
            tc.tile_pool(name="cxps", bufs=2, space="PSUM") as cxps,
        ):
            # ---- constants ----
            bq_t = consts.tile([128, NKC], F32, tag="bq")
            bk_t = consts.tile([128, NKC], F32, tag="bk")
            nc.sync.dma_start(out=bq_t, in_=bq2[:, :])
            nc.sync.dma_start(out=bk_t, in_=bk2[:, :])
            eps_t = consts.tile([128, 1], F32, tag="eps")
            nc.vector.memset(eps_t, EPS)
            ones16f = consts.tile([128, 16], F32, tag="o16f")
            nc.vector.memset(ones16f, 1.0)
            ones16r = consts.tile([128, 16], F32R, tag="o16r")
            nc.vector.tensor_copy(ones16r[:, :], ones16f[:, :])

            ktdram = dramp.tile([NHP, 128, S], F32R, tag="ktd")

            def stage_row_bcast(param, pool, tag):
                t = pool.tile([128, D], F32, tag=tag)
                rw = rowp.tile([1, D], F32, tag="row")
                nc.sync.dma_start(out=rw, in_=param[:, :])
                nc.gpsimd.partition_broadcast(t[:, :], rw[:, :])
                return t

            def load_wslice(wparam, kc, hp, pool, round_eng):
                st = stage.tile([128, 128], F32, tag="wst")
                nc.sync.dma_start(
                    out=st,
                    in_=wparam[kc * 128 : (kc + 1) * 128, hp * 128 : (hp + 1) * 128],
                )
                wr = pool.tile([128, 128], F32R, tag="wsl")
                if round_eng == "v":
                    nc.vector.tensor_copy(wr[:, :], st[:, :])
                else:
                    nc.gpsimd.tensor_copy(out=wr[:, :], in_=st[:, :])
                return wr

            _pbs = [0]

            def proj_banks(name):
                r = _pbs[0] % 3
                _pbs[0] += 1
                if r == 0:
                    a = pp.tile([128, 512], F32, tag="pp", name=f"{name}a")
                    b = pp.tile([128, 512], F32, tag="pp", name=f"{name}b")
                    return a[:, :], b[:, :]
                pair = stpp.tile([128, 2, RB], F32, tag="stp", name=f"{name}p")
                return pair[:, 0, :], pair[:, 1, :]

            # ================= Q projection =================
            # Qt[hp] = [128(d of 2 heads), 512(i)] = Wq[dslice,:] @ q_rows^T
            # head-pair groups interleaved in bank pairs
            Qt = [None] * NHP
            with (
                tc.tile_pool(name="xq", bufs=1) as xqpool,
                tc.tile_pool(name="wsl", bufs=32) as wslpool,
            ):
                xq_r = []
                for kc in range(NKC):
                    st = stage.tile([128, RB], F32, tag="stg")
                    nc.sync.dma_start(out=st, in_=xqT[kc * 128 : (kc + 1) * 128, :])
                    xr = xqpool.tile([128, RB], F32R, tag=f"xq{kc}")
                    nc.vector.tensor_copy(xr[:, :], st[:, :])
                    xq_r.append(xr)
                wslq = {
                    hp: [load_wslice(wqT, kc, hp, wslpool, "v") for kc in range(NKC)]
                    for hp in (0, 1)
                }
                for hpb in range(NHP // 2):
                    hps = (2 * hpb, 2 * hpb + 1)
                    if hpb + 1 < NHP // 2:
                        for hp in (2 * hpb + 2, 2 * hpb + 3):
                            wslq[hp] = [
                                load_wslice(wqT, kc, hp, wslpool, "v")
                                for kc in range(NKC)
                            ]
                    wsl = wslq
                    _a, _b = proj_banks(f"psq{hpb}")
                    pss = {hps[0]: _a, hps[1]: _b}
                    for kc in range(NKC):
                        for hp in hps:
                            nc.tensor.matmul(
                                pss[hp][:, :],
                                wsl[hp][kc][:, :],
                                xq_r[kc][:, :],
                                start=(kc == 0),
                                stop=(kc == NKC - 1),
                            )
                    for hp in hps:
                        qt = qtp.tile([128, RB], F32R, tag=f"qt{hp}")
                        nc.vector.tensor_scalar(
                            qt[:, :], pss[hp][:, :], bq_t[:, hp : hp + 1], None, ALU.add
                        )
                        Qt[hp] = qt

            # ================= V projection =================
            # V in natural [j, d] layout + ones column: vp[jc]=[128(j),16(h),65]
            # per jc: the two dc-halves form an interleaved bank pair
            Vp = []
            for jc in range(NJC):
                vp = vpp.tile([128, H, HD + 1], F32R, tag=f"vp{jc}")
                nc.vector.tensor_copy(vp[:, :, HD], ones16r[:, :])
                Vp.append(vp)
            with (
                tc.tile_pool(name="wv", bufs=1) as wvpool,
                tc.tile_pool(name="xvsl", bufs=16) as xvslpool,
                tc.tile_pool(name="bvp", bufs=1) as bvpool,
            ):
                bv_bc = stage_row_bcast(bvr, bvpool, "bvbc")
                wv_r = []
                for kc in range(NKC):
                    wr = wvpool.tile([128, D], F32R, tag=f"wv{kc}")
                    for piece in range(2):
                        st = stage.tile([128, 512], F32, tag="stg")
                        nc.sync.dma_start(
                            out=st,
                            in_=wvT[
                                kc * 128 : (kc + 1) * 128,
                                piece * 512 : (piece + 1) * 512,
                            ],
                        )
                        nc.scalar.activation(
                            wr[:, piece * 512 : (piece + 1) * 512], st[:, :], AF.Copy
                        )
                    wv_r.append(wr)
                def load_xvsl(jc):
                    out = []
                    for kc in range(NKC):
                        st = stage.tile([128, 128], F32, tag="wst")
                        nc.sync.dma_start(
                            out=st,
                            in_=xvT[
                                kc * 128 : (kc + 1) * 128, jc * 128 : (jc + 1) * 128
                            ],
                        )
                        xr = xvslpool.tile([128, 128], F32R, tag="xvsl")
                        nc.vector.tensor_copy(xr[:, :], st[:, :])
                        out.append(xr)
                    return out

                xvnext = load_xvsl(0)
                for jc in range(NJC):
                    xvsl = xvnext
                    if jc + 1 < NJC:
                        xvnext = load_xvsl(jc + 1)
                    pss = list(proj_banks(f"psv{jc}"))
                    for kc in range(NKC):
                        for dc in range(2):
                            nc.tensor.matmul(
                                pss[dc][:, :],
                                xvsl[kc][:, :],
                                wv_r[kc][:, dc * 512 : (dc + 1) * 512],
                                start=(kc == 0),
                                stop=(kc == NKC - 1),
                            )
                    for dc in range(2):
                        nc.vector.tensor_tensor(
                            Vp[jc][:, dc * 8 : (dc + 1) * 8, 0:HD],
                            pss[dc][:, :].rearrange("p (h d) -> p h d", h=8),
                            bv_bc[:, dc * 512 : (dc + 1) * 512].rearrange(
                                "p (h d) -> p h d", h=8
                            ),
                            ALU.add,
                        )

            # ================= K projection (spilled to HBM) =================
            with (
                tc.tile_pool(name="xk", bufs=1) as xkpool,
                tc.tile_pool(name="wslk", bufs=16) as wslkpool,
                tc.tile_pool(name="kev", bufs=2) as kevpool,
            ):
                xk_r = []
                for kc in range(NKC):
                    xr = xkpool.tile([128, S], F32R, tag=f"xk{kc}")
                    for piece in range(4):
                        st = stage.tile([128, 512], F32, tag="stg")
                        nc.sync.dma_start(
                            out=st,
                            in_=xkT[
                                kc * 128 : (kc + 1) * 128,
                                piece * 512 : (piece + 1) * 512,
                            ],
                        )
                        if piece % 2 == 0:
                            nc.vector.tensor_copy(
                                xr[:, piece * 512 : (piece + 1) * 512], st[:, :]
                            )
                        else:
                            nc.scalar.activation(
                                xr[:, piece * 512 : (piece + 1) * 512],
                                st[:, :],
                                AF.Copy,
                            )
                    xk_r.append(xr)
                wslk = {
                    0: [load_wslice(wkT, kc, 0, wslkpool, "v") for kc in range(NKC)]
                }
                for hp in range(NHP):
                    if hp + 1 < NHP:
                        wslk[hp + 1] = [
                            load_wslice(wkT, kc, hp + 1, wslkpool, "v")
                            for kc in range(NKC)
                        ]
                    wsl = wslk.pop(hp)
                    # interleave the 4 j4-groups as 2 bank pairs
                    for j4p in range(2):
                        j4s = (2 * j4p, 2 * j4p + 1)
                        _a, _b = proj_banks(f"psk{hp}_{j4p}")
                        pss = {j4s[0]: _a, j4s[1]: _b}
                        for kc in range(NKC):
                            for j4 in j4s:
                                nc.tensor.matmul(
                                    pss[j4][:, :],
                                    wsl[kc][:, :],
                                    xk_r[kc][:, j4 * 512 : (j4 + 1) * 512],
                                    start=(kc == 0),
                                    stop=(kc == NKC - 1),
                                )
                        for j4 in j4s:
                            kev = kevpool.tile([128, 512], F32R, tag="kev")
                            nc.vector.tensor_scalar(
                                kev[:, :],
                                pss[j4][:, :],
                                bk_t[:, hp : hp + 1],
                                None,
                                ALU.add,
                            )
                            nc.sync.dma_start(
                                out=ktdram[hp, :, j4 * 512 : (j4 + 1) * 512],
                                in_=kev[:, :],
                            )

            # ================= attention =================
            # software pipeline over heads: PV chain of head i-1 is interleaved
            # with the scores/exp stream of head i (alternating PSUM banks).
            CtxP = [None] * NHP
            with (
                tc.tile_pool(name="ktr", bufs=2) as ktrpool,
                tc.tile_pool(name="pt", bufs=(5 if masked else 6)) as ptp,
                tc.tile_pool(name="lin", bufs=2) as linp,
                tc.tile_pool(name="mbp", bufs=1) as mbp,
            ):
                mb = None
                if masked:
                    mb = []
                    for jc in range(NJC):
                        mi = stage.tile([128, RB], I32, tag="stg")
                        nc.sync.dma_start(
                            out=mi, in_=mskT[jc * 128 : (jc + 1) * 128, :]
                        )
                        mt = mbp.tile([128, RB], BF16, tag=f"mb{jc}")
                        nc.vector.tensor_scalar(
                            mt[:, :], mi[:, :], 1e9, 1e9, ALU.mult, ALU.subtract
                        )
                        mb.append(mt)

                def load_ktr(hp):
                    ktr = ktrpool.tile([128, S], F32R, tag="ktr")
                    for piece in range(4):
                        st = stage.tile([128, 512], F32, tag="stg")
                        nc.scalar.dma_start(
                            out=st,
                            in_=ktdram[hp, :, piece * 512 : (piece + 1) * 512].bitcast(
                                F32
                            ),
                        )
                        nc.gpsimd.tensor_copy(
                            out=ktr[:, piece * 512 : (piece + 1) * 512],
                            in_=st[:, :],
                        )
                    return ktr

                def emit_tail(state):
                    """l -> 1/l -> broadcast; normalize ctx + probs; DMA out."""
                    h, po, cps, pts, ctx_packed = state
                    linv = small.tile([1, RB], F32, tag="linv")
                    nc.vector.reciprocal(linv[:, :], cps[HD : HD + 1, :])
                    linb = linp.tile([128, RB], F32, tag="linb")
                    nc.gpsimd.partition_broadcast(linb[:, :], linv[:, :])
                    nc.vector.tensor_mul(
                        ctx_packed[po : po + 64, :], cps[0:HD, :], linb[0:HD, :]
                    )
                    for jq in range(NJC // 4):
                        pn = pnp.tile([128, 4, RB], F32, tag="pn")
                        eng = nc.gpsimd if (jq == 1) else nc.vector
                        eng.tensor_mul(
                            pn[:, :, :],
                            pts[jq][:, :, :].bitcast(F32),
                            linb[:, :].rearrange("p (u i) -> p u i", u=1).broadcast_to(
                                [128, 4, RB]
                            ),
                        )
                        nc.sync.dma_start(
                            out=attnT[h, jq * 512 : (jq + 1) * 512, :].rearrange(
                                "(u p) i -> p u i", u=4
                            ),
                            in_=pn[:, :, :],
                        )

                NP2 = NJC // 2  # 8 jc-pairs per head
                ktr_tiles = {}
                for idx in range(H):
                    hp, h2 = divmod(idx, 2)
                    if h2 == 0:
                        if hp == 0:
                            ktr_tiles[0] = load_ktr(0)
                        ctx_packed = cxp.tile([128, RB], F32R, tag=f"cxp{hp}")
                        CtxP[hp] = ctx_packed
                    ktr = ktr_tiles[hp]
                    po = h2 * 64
                    cps = cxps.tile([128, RB], F32, tag="cxps")
                    pts = []  # pt pair tiles

                    def pv(jp):
                        jq, half = divmod(jp, 2)
                        for sub in range(2):
                            jc = 2 * jp + sub
                            nc.tensor.matmul(
                                cps[0 : HD + 1, :],
                                Vp[jc][:, idx, :],
                                pts[jq][:, 2 * half + sub, :],
                                start=(jc == 0),
                                stop=(jc == NJC - 1),
                            )

                    for jp in range(NP2):
                        jq, half = divmod(jp, 2)
                        if half == 0:
                            ptq = ptp.tile([128, 4, RB], F32R, tag="pt", name="ptq")
                            pts.append(ptq)
                        st_ps = stpp.tile([128, 2, RB], F32, tag="stp")
                        for sub in range(2):
                            jc = 2 * jp + sub
                            nc.tensor.matmul(
                                st_ps[:, sub, :],
                                ktr[po : po + 64, jc * 128 : (jc + 1) * 128],
                                Qt[hp][po : po + 64, :],
                                start=True,
                                stop=True,
                            )
                        if masked:
                            for sub in range(2):
                                nc.vector.tensor_tensor(
                                    st_ps[:, sub, :],
                                    st_ps[:, sub, :],
                                    mb[2 * jp + sub][:, :],
                                    ALU.add,
                                )
                        nc.scalar.activation(
                            pts[jq][:, 2 * half : 2 * half + 2, :],
                            st_ps[:, :, :],
                            AF.Exp,
                            scale=0.125,
                        )
                        if jp >= 1:
                            pv(jp - 1)
                    pv(NP2 - 1)
                    emit_tail((idx, po, cps, pts, CtxP[hp]))
                    # prefetch next pair's K mid-pair
                    if h2 == 0 and hp + 1 < NHP:
                        ktr_tiles[hp + 1] = load_ktr(hp + 1)

            # ================= output proj + layernorm =================
            with (
                tc.tile_pool(name="wo", bufs=1) as wopool,
                tc.tile_pool(name="bcs", bufs=1) as bcpool,
                tc.tile_pool(name="xt", bufs=2) as xtpool,
            ):
                bo_bc = stage_row_bcast(bor, bcpool, "bobc")
                gm_bc = stage_row_bcast(gmr, bcpool, "gmbc")
                bt_bc = stage_row_bcast(btr, bcpool, "btbc")
                wo_r = []
                for kc in range(NKC):
                    wr = wopool.tile([128, D], F32R, tag=f"wo{kc}")
                    for piece in range(2):
                        st = stage.tile([128, 512], F32, tag="stg")
                        nc.scalar.dma_start(
                            out=st,
                            in_=woT[
                                kc * 128 : (kc + 1) * 128,
                                piece * 512 : (piece + 1) * 512,
                            ],
                        )
                        nc.scalar.activation(
                            wr[:, piece * 512 : (piece + 1) * 512], st[:, :], AF.Copy
                        )
                    wo_r.append(wr)

                for ic in range(NIC):
                    xt = xtpool.tile([128, D], F32, tag="xt")
                    pss = list(proj_banks(f"psv{jc}"))
                    for hp in range(NHP):
                        for dc in range(2):
                            nc.tensor.matmul(
                                pss[dc][:, :],
                                CtxP[hp][:, ic * 128 : (ic + 1) * 128],
                                wo_r[hp][:, dc * 512 : (dc + 1) * 512],
                                start=(hp == 0),
                                stop=(hp == NHP - 1),
                            )
                    for dc in range(2):
                        rst = stage.tile([128, 512], F32, tag="stg")
                        nc.sync.dma_start(
                            out=rst,
                            in_=res[
                                ic * 128 : (ic + 1) * 128, dc * 512 : (dc + 1) * 512
                            ],
                        )
                        nc.vector.tensor_tensor(
                            xt[:, dc * 512 : (dc + 1) * 512],
                            pss[dc][:, :],
                            rst[:, :],
                            ALU.add,
                        )
                    nc.vector.tensor_tensor(xt[:, :], xt[:, :], bo_bc[:, :], ALU.add)
                    stats = small.tile([128, 2, 6], F32, tag="stats")
                    for sg in range(2):
                        nc.vector.bn_stats(
                            stats[:, sg, :], xt[:, sg * 512 : (sg + 1) * 512]
                        )
                    mv = small.tile([128, 2], F32, tag="mv")
                    nc.vector.bn_aggr(mv[:, :], stats[:, :, :])
                    sd = small.tile([128, 1], F32, tag="sd")
                    nc.scalar.activation(
                        sd[:, :], mv[:, 1:2], AF.Sqrt, bias=eps_t[:, 0:1]
                    )
                    rstd = small.tile([128, 1], F32, tag="rstd")
                    nc.vector.reciprocal(rstd[:, :], sd[:, :])
                    nc.vector.tensor_scalar(
                        xt[:, :],
                        xt[:, :],
                        mv[:, 0:1],
                        rstd[:, 0:1],
                        ALU.subtract,
                        ALU.mult,
                    )
                    nc.vector.tensor_tensor(xt[:, :], xt[:, :], gm_bc[:, :], ALU.mult)
                    nc.vector.tensor_tensor(xt[:, :], xt[:, :], bt_bc[:, :], ALU.add)
                    nc.sync.dma_start(
                        out=outp[ic * 128 : (ic + 1) * 128, :], in_=xt[:, :]
                    )

    nc.finalize()
    return nc


def get_nc(masked: bool):
    if masked not in _nc_cache:
        _nc_cache[masked] = _build(masked)
    return _nc_cache[masked]


def make_in_maps(q, k, v, mask, Wq, bq, Wk, bk, Wv, bv, Wo, bo, gamma, beta, masked):
    f32 = np.float32
    kT = [np.ascontiguousarray(k[b].T).astype(f32, copy=False) for b in range(B)]
    vT = [np.ascontiguousarray(v[b].T).astype(f32, copy=False) for b in range(B)]
    wqT = np.ascontiguousarray(np.asarray(Wq, f32).T)
    wkT = np.ascontiguousarray(np.asarray(Wk, f32).T)
    wvT = np.ascontiguousarray(np.asarray(Wv, f32).T)
    woT = np.ascontiguousarray(np.asarray(Wo, f32).T)
    bq2 = np.ascontiguousarray(np.asarray(bq, f32).reshape(NKC, 128).T)
    bk2 = np.ascontiguousarray(np.asarray(bk, f32).reshape(NKC, 128).T)
    bvr = np.ascontiguousarray(np.asarray(bv, f32).reshape(1, D))
    bor = np.ascontiguousarray(np.asarray(bo, f32).reshape(1, D))
    gmr = np.ascontiguousarray(np.asarray(gamma, f32).reshape(1, D))
    btr = np.ascontiguousarray(np.asarray(beta, f32).reshape(1, D))
    in_maps = []
    for c in range(NCORES):
        b, r = divmod(c, NCORES // B)
        rows = slice(r * RB, (r + 1) * RB)
        m = {
            "xqT": np.ascontiguousarray(q[b, rows, :].T),
            "res": np.ascontiguousarray(q[b, rows, :]),
            "xkT": kT[b],
            "xvT": vT[b],
            "wqT": wqT,
            "wkT": wkT,
            "wvT": wvT,
            "woT": woT,
            "bq2": bq2,
            "bk2": bk2,
            "bvr": bvr,
            "bor": bor,
            "gmr": gmr,
            "btr": btr,
        }
        if masked:
            m["mskT"] = np.ascontiguousarray(mask[b, rows, :].T).astype(
                np.int32, copy=False
            )
        in_maps.append(m)
    return in_maps


def assemble(results):
    output = np.empty((B, S, D), np.float32)
    attention = np.empty((B, H, S, S), np.float32)
    for c in range(NCORES):
        b, r = divmod(c, NCORES // B)
        rows = slice(r * RB, (r + 1) * RB)
        output[b, rows, :] = results[c]["outp"]
        attention[b, :, rows, :] = results[c]["attnT"].transpose(0, 2, 1)
    return output, attention


def kernel(q, k, v, mask, Wq, bq, Wk, bk, Wv, bv, Wo, bo, gamma, beta, **run_kwargs):
    q = np.ascontiguousarray(np.asarray(q), dtype=np.float32)
    k = np.ascontiguousarray(np.asarray(k), dtype=np.float32)
    v = np.ascontiguousarray(np.asarray(v), dtype=np.float32)
    mask = np.asarray(mask)
    masked = bool((np.asarray(mask) == 0).any())
    nc = get_nc(masked)
    in_maps = make_in_maps(
        q, k, v, mask, Wq, bq, Wk, bk, Wv, bv, Wo, bo, gamma, beta, masked
    )
    res = run_bass_kernel_spmd(nc, in_maps, list(range(NCORES)), **run_kwargs)
    output, attention = assemble(res.results)
    if run_kwargs:
        return (output, attention), res
    return output, attention
